# revision 2
# baseline (speedup 1.0000x reference)
"""DeepseekMoE layer on 8 Trainium2 NeuronCores (Bass/Tile, expert-parallel).

Sharding (per the expert-parallel hint):
  - 16 routed experts -> 2 per core; token dispatch (all-to-all) is emulated at
    the sharding layer: the host computes the discrete top-4 routing, gathers
    each expert's tokens into a compact transposed batch, and scatter-adds the
    compact expert outputs back into the full output ("combine").
  - Shared expert is tensor-parallel over its intermediate dim (2816/8 = 352
    columns per core); the 8 partial outputs are summed on gather.
  - Gate (softmax + renormalized top-4 combine weights) is replicated and
    computed ON DEVICE from the hidden states; the host only supplies the
    discrete 0/1 top-4 mask (routing decision) and gather indices.

All FLOPs that produce output values run on device. Matmuls use float32r
(full-rate fp32 mode, ~1.5e-4 rel-rms) except the tiny gate matmul which uses
exact 2-pass fp32.
"""

import os
import numpy as np

H = 2048          # hidden size
E = 16            # routed experts
TOPK = 4
I = 1408          # routed expert intermediate
ISH = 2816        # shared expert intermediate
T = 1024          # tokens
P = 128
NCORES = 8
EPC = 2           # experts per core
ISS = ISH // NCORES                  # 352 shared columns per core
KH = H // P                          # 16 k-tiles over H
MI = I // P                          # 11 m-tiles over I
MH = H // P                          # 16 m-tiles over H
KI = I // P                          # 11 k-tiles over I
ZERO_ROW_FLAT = T * E                # flat index of the zeroed scratch row

_NC_CACHE = {}
LAST_RESULTS = None  # BassKernelResults of the most recent run (for test.py)


def _token_chunks(C):
    """Split [0, C) into matmul moving-dim chunks of <=512."""
    out = []
    off = 0
    while off < C:
        sz = min(512, C - off)
        out.append((off, sz))
        off += sz
    return out


def _shared_m_tiles():
    """(offset, size) tiles over the 352-wide shared slice."""
    out = []
    off = 0
    while off < ISS:
        sz = min(P, ISS - off)
        out.append((off, sz))
        off += sz
    return out


def _build(C):
    import concourse.bacc as bacc
    import concourse.bass as bass
    import concourse.mybir as mybir
    import concourse.tile as tile
    from concourse.masks import make_identity

    f32 = mybir.dt.float32
    f32r = mybir.dt.float32r
    i32 = mybir.dt.int32
    SILU = mybir.ActivationFunctionType.Silu
    EXP = mybir.ActivationFunctionType.Exp
    X = mybir.AxisListType.X

    CH = _token_chunks(C)
    SMT = _shared_m_tiles()
    C2 = EPC * C

    nc = bacc.Bacc("TRN2", target_bir_lowering=False, debug=False)

    xt_h = nc.dram_tensor("xt", [H, T], f32r, kind="ExternalInput")
    gwt_h = nc.dram_tensor("gwt", [H, E], f32, kind="ExternalInput")
    mask_h = nc.dram_tensor("mask", [T, E], f32, kind="ExternalInput")
    xg_h = nc.dram_tensor("xg", [H, C2], f32r, kind="ExternalInput")
    widx_h = nc.dram_tensor("widx", [C2, 1], i32, kind="ExternalInput")
    wg_h = [nc.dram_tensor(f"wg{j}", [H, I], f32r, kind="ExternalInput") for j in range(EPC)]
    wu_h = [nc.dram_tensor(f"wu{j}", [H, I], f32r, kind="ExternalInput") for j in range(EPC)]
    wd_h = [nc.dram_tensor(f"wd{j}", [I, H], f32r, kind="ExternalInput") for j in range(EPC)]
    swg_h = nc.dram_tensor("swg", [H, ISS], f32r, kind="ExternalInput")
    swu_h = nc.dram_tensor("swu", [H, ISS], f32r, kind="ExternalInput")
    swd_h = nc.dram_tensor("swd", [ISS, H], f32r, kind="ExternalInput")
    zt_h = nc.dram_tensor("zt", [H, C2], f32, kind="ExternalOutput")
    st_h = nc.dram_tensor("st", [H, T], f32, kind="ExternalOutput")

    with tile.TileContext(nc) as tc:
        with (
            tc.tile_pool(name="resident", bufs=1) as res_pool,
            tc.tile_pool(name="xgp", bufs=1) as xg_pool,
            tc.tile_pool(name="acts", bufs=1) as act_pool,
            tc.tile_pool(name="wstream", bufs=24) as wst_pool,
            tc.tile_pool(name="sstream", bufs=16) as sst_pool,
            tc.tile_pool(name="small", bufs=2) as small_pool,
            tc.tile_pool(name="stage", bufs=3) as stage_pool,
            tc.tile_pool(name="ps", bufs=1, space="PSUM") as ps_pool,
            tc.tile_pool(name="dram", bufs=1, space="DRAM") as dram_pool,
        ):
            # ---------------- resident loads ----------------
            xt_t = [res_pool.tile([P, T], f32r, name=f"xt{k}", tag=f"xt{k}") for k in range(KH)]
            for k in range(KH):
                nc.sync.dma_start(xt_t[k][:], xt_h[k * P:(k + 1) * P, :])
            gwt_t = [res_pool.tile([P, E], f32, name=f"gwt{k}", tag=f"gwt{k}") for k in range(KH)]
            for k in range(KH):
                nc.sync.dma_start(gwt_t[k][:], gwt_h[k * P:(k + 1) * P, :])
            mask_t = [res_pool.tile([P, E], f32, name=f"mask{t8}", tag=f"mask{t8}") for t8 in range(T // P)]
            for t8 in range(T // P):
                nc.sync.dma_start(mask_t[t8][:], mask_h[t8 * P:(t8 + 1) * P, :])
            ident = res_pool.tile([P, P], f32, name="ident", tag="ident")
            make_identity(nc, ident[:])
            zbias = res_pool.tile([P, 1], f32, name="zbias", tag="zbias")
            nc.vector.memset(zbias[:], 0.0)

            # combine-weight scratch in HBM: rows 0..T-1 = combine, row T = zeros
            wflat = dram_pool.tile([(T + 1) * E, 1], f32, name="wflat")
            wflat2d = wflat[:].rearrange("(a b) o -> a (b o)", b=E)
            zrow = res_pool.tile([1, E], f32, name="zrow", tag="zrow")
            nc.vector.memset(zrow[:], 0.0)
            nc.sync.dma_start(wflat2d[T:T + 1, :], zrow[:])

            # ---------------- gate (exact fp32) ----------------
            lgps = ps_pool.tile([E, T], f32, name="lgps", tag="B1", bufs=2)
            for n in range(T // 512):
                for k in range(KH):
                    nc.tensor.matmul(
                        lgps[:, n * 512:(n + 1) * 512],
                        lhsT=gwt_t[k][:],
                        rhs=xt_t[k][:, n * 512:(n + 1) * 512].bitcast(f32),
                        start=(k == 0), stop=(k == KH - 1),
                    )
            lgsb = res_pool.tile([E, T], f32, name="lgsb", tag="lgsb")
            nc.scalar.copy(lgsb[:], lgps[:])
            for t8 in range(T // P):
                trps = ps_pool.tile([P, E], f32, name=f"tr{t8}", tag="A1", bufs=4)
                nc.tensor.transpose(
                    out=trps[:], in_=lgsb[:, t8 * P:(t8 + 1) * P], identity=ident[0:E, 0:E],
                )
                sc = small_pool.tile([P, E], f32, name=f"sc{t8}", tag="sc")
                nc.scalar.activation(sc[:], trps[:], EXP, bias=zbias[:])
                mskd = small_pool.tile([P, E], f32, name=f"mskd{t8}", tag="mskd")
                nc.vector.tensor_mul(out=mskd[:], in0=sc[:], in1=mask_t[t8][:])
                ssum = small_pool.tile([P, 1], f32, name=f"ssum{t8}", tag="ssum")
                nc.vector.reduce_sum(ssum[:], mskd[:], axis=X)
                rsum = small_pool.tile([P, 1], f32, name=f"rsum{t8}", tag="rsum")
                nc.vector.reciprocal(rsum[:], ssum[:])
                comb = small_pool.tile([P, E], f32, name=f"comb{t8}", tag="comb")
                nc.vector.tensor_scalar_mul(comb[:], mskd[:], rsum[:, :1])
                nc.sync.dma_start(wflat2d[t8 * P:(t8 + 1) * P, :], comb[:])

            # ---------------- per-expert routed MLPs ----------------
            wb = [res_pool.tile([P, C], f32, name=f"wb{j}", tag=f"wb{j}") for j in range(EPC)]
            for j in range(EPC):
                # gather this expert's per-slot combine weights and broadcast
                # them across partitions: wb[j][p, c] = w_slot[c] for all p
                off = 0
                while off < C:
                    csz = min(P, C - off)
                    it = small_pool.tile([P, 1], i32, name=f"it{j}_{off}", tag="it")
                    nc.sync.dma_start(it[:csz], widx_h[j * C + off:j * C + off + csz, :])
                    wslot = small_pool.tile([P, 1], f32, name=f"ws{j}_{off}", tag="ws")
                    nc.gpsimd.indirect_dma_start(
                        out=wslot[:csz, :], out_offset=None, in_=wflat[:],
                        in_offset=bass.IndirectOffsetOnAxis(ap=it[:csz, :1], axis=0),
                    )
                    wbps = ps_pool.tile([P, P], f32, name=f"wbps{j}_{off}", tag="A1", bufs=4)
                    nc.tensor.transpose(
                        out=wbps[:, :csz],
                        in_=wslot[:csz, :1].to_broadcast([csz, P]),
                        identity=ident[0:csz, 0:csz],
                    )
                    nc.vector.tensor_copy(wb[j][:, off:off + csz], wbps[:, :csz])
                    off += csz

                # gathered activations for this expert: xg columns [j*C, (j+1)*C)
                xg_t = [xg_pool.tile([P, C], f32r, name=f"xg{j}_{k}", tag=f"xg{k}") for k in range(KH)]
                for k in range(KH):
                    nc.sync.dma_start(xg_t[k][:], xg_h[k * P:(k + 1) * P, j * C:(j + 1) * C])

                # --- up/gate projections + SwiGLU (feature-major: [I-tile, C]) ---
                g_t = [act_pool.tile([P, C], f32, name=f"g{j}_{m}", tag=f"g{m}") for m in range(MI)]
                a_t = [act_pool.tile([P, C], f32r, name=f"a{j}_{m}", tag=f"a{m}") for m in range(MI)]
                for m in range(MI):
                    for (coff, csz) in CH:
                        psg = ps_pool.tile([P, csz], f32, name=f"psg{j}_{m}_{coff}", tag="A1", bufs=4)
                        for k in range(KH):
                            wt = wst_pool.tile([P, P], f32r, name=f"twg{j}_{m}_{coff}_{k}", tag="wst")
                            nc.sync.dma_start(wt[:], wg_h[j][k * P:(k + 1) * P, m * P:(m + 1) * P])
                            nc.tensor.matmul(psg[:], lhsT=wt[:], rhs=xg_t[k][:, coff:coff + csz],
                                             start=(k == 0), stop=(k == KH - 1))
                        nc.scalar.activation(g_t[m][:, coff:coff + csz], psg[:], SILU, bias=zbias[:])
                        psu = ps_pool.tile([P, csz], f32, name=f"psu{j}_{m}_{coff}", tag="A1", bufs=4)
                        for k in range(KH):
                            wt = wst_pool.tile([P, P], f32r, name=f"twu{j}_{m}_{coff}_{k}", tag="wst")
                            nc.sync.dma_start(wt[:], wu_h[j][k * P:(k + 1) * P, m * P:(m + 1) * P])
                            nc.tensor.matmul(psu[:], lhsT=wt[:], rhs=xg_t[k][:, coff:coff + csz],
                                             start=(k == 0), stop=(k == KH - 1))
                        # silu(g)*u straight out of PSUM, then scale by the
                        # per-token combine weight; write rounded to f32r
                        nc.vector.tensor_mul(out=g_t[m][:, coff:coff + csz],
                                             in0=g_t[m][:, coff:coff + csz], in1=psu[:])
                    nc.vector.tensor_mul(out=a_t[m][:], in0=g_t[m][:], in1=wb[j][:])

                # --- down projection: zt[m-tile, slots] = sum_k WdT a ---
                for m in range(MH):
                    for (coff, csz) in CH:
                        psz = ps_pool.tile([P, csz], f32, name=f"psz{j}_{m}_{coff}", tag="A1", bufs=4)
                        for k in range(KI):
                            wt = wst_pool.tile([P, P], f32r, name=f"twd{j}_{m}_{coff}_{k}", tag="wst")
                            nc.sync.dma_start(wt[:], wd_h[j][k * P:(k + 1) * P, m * P:(m + 1) * P])
                            nc.tensor.matmul(psz[:], lhsT=wt[:], rhs=a_t[k][:, coff:coff + csz],
                                             start=(k == 0), stop=(k == KI - 1))
                        zst = stage_pool.tile([P, csz], f32, name=f"zst{j}_{m}_{coff}", tag="zst")
                        nc.scalar.copy(zst[:], psz[:])
                        nc.sync.dma_start(zt_h[m * P:(m + 1) * P, j * C + coff:j * C + coff + csz], zst[:])

            # ---------------- shared expert (TP slice over ISS) ----------------
            sg_t = [act_pool.tile([P, T], f32, name=f"sg{m}", tag=f"sg{m}") for m in range(len(SMT))]
            as_t = [act_pool.tile([P, T], f32r, name=f"as{m}", tag=f"as{m}") for m in range(len(SMT))]
            for mi, (moff, msz) in enumerate(SMT):
                psgs = ps_pool.tile([msz, T], f32, name=f"psgs{mi}", tag="B1", bufs=2)
                for k in range(KH):
                    wt = sst_pool.tile([P, P], f32r, name=f"tsg{mi}_{k}", tag="sst")
                    nc.sync.dma_start(wt[:, :msz], swg_h[k * P:(k + 1) * P, moff:moff + msz])
                    for n in range(T // 512):
                        nc.tensor.matmul(psgs[:, n * 512:(n + 1) * 512],
                                         lhsT=wt[:, :msz], rhs=xt_t[k][:, n * 512:(n + 1) * 512],
                                         start=(k == 0), stop=(k == KH - 1))
                nc.scalar.activation(sg_t[mi][:msz, :], psgs[:], SILU, bias=zbias[:msz])
                psus = ps_pool.tile([msz, T], f32, name=f"psus{mi}", tag="B1", bufs=2)
                for k in range(KH):
                    wt = sst_pool.tile([P, P], f32r, name=f"tsu{mi}_{k}", tag="sst")
                    nc.sync.dma_start(wt[:, :msz], swu_h[k * P:(k + 1) * P, moff:moff + msz])
                    for n in range(T // 512):
                        nc.tensor.matmul(psus[:, n * 512:(n + 1) * 512],
                                         lhsT=wt[:, :msz], rhs=xt_t[k][:, n * 512:(n + 1) * 512],
                                         start=(k == 0), stop=(k == KH - 1))
                nc.vector.tensor_mul(out=as_t[mi][:msz, :], in0=sg_t[mi][:msz, :], in1=psus[:])

            for m in range(MH):
                psys = ps_pool.tile([P, T], f32, name=f"psys{m}", tag="B1", bufs=2)
                for ki, (koff, ksz) in enumerate(SMT):
                    wt = sst_pool.tile([P, P], f32r, name=f"tsd{m}_{ki}", tag="sst")
                    nc.sync.dma_start(wt[:ksz, :], swd_h[koff:koff + ksz, m * P:(m + 1) * P])
                    for n in range(T // 512):
                        nc.tensor.matmul(psys[:, n * 512:(n + 1) * 512],
                                         lhsT=wt[:ksz, :], rhs=as_t[ki][:ksz, n * 512:(n + 1) * 512],
                                         start=(ki == 0), stop=(ki == len(SMT) - 1))
                sst = stage_pool.tile([P, T], f32, name=f"sst{m}", tag="sstage")
                nc.scalar.copy(sst[:], psys[:])
                nc.sync.dma_start(st_h[m * P:(m + 1) * P, :], sst[:])

    nc.compile()
    return nc


def _get_nc(C):
    if C not in _NC_CACHE:
        _NC_CACHE[C] = _build(C)
    return _NC_CACHE[C]


def kernel(**inputs):
    global LAST_RESULTS
    from concourse.bass_utils import run_bass_kernel_spmd

    hs = np.asarray(inputs["hidden_states"], dtype=np.float32)
    gate_w = np.asarray(inputs["gate_w"], dtype=np.float32)
    w_gate = np.asarray(inputs["w_gate"], dtype=np.float32)
    w_up = np.asarray(inputs["w_up"], dtype=np.float32)
    w_down = np.asarray(inputs["w_down"], dtype=np.float32)
    sw_gate = np.asarray(inputs["sw_gate"], dtype=np.float32)
    sw_up = np.asarray(inputs["sw_up"], dtype=np.float32)
    sw_down = np.asarray(inputs["sw_down"], dtype=np.float32)

    orig_shape = hs.shape
    x = hs.reshape(-1, H)
    assert x.shape[0] == T

    # ---- host: discrete routing only (top-4 selection + dispatch tables) ----
    logits = x @ gate_w.T
    smax = logits.max(axis=-1, keepdims=True)
    sc = np.exp(logits - smax)
    sc /= sc.sum(axis=-1, keepdims=True)
    order = np.argsort(-sc, axis=-1, kind="stable")[:, :TOPK]
    mask = np.zeros((T, E), dtype=np.float32)
    mask[np.arange(T)[:, None], order] = 1.0
    tok_lists = [np.nonzero(mask[:, e])[0].astype(np.int64) for e in range(E)]
    maxn = max(len(tk) for tk in tok_lists)
    C = max(64, int(np.ceil(maxn / 64)) * 64)

    nc = _get_nc(C)

    xT = np.ascontiguousarray(x.T)
    gwt = np.ascontiguousarray(gate_w.T)

    in_maps = []
    for c in range(NCORES):
        es = [EPC * c + j for j in range(EPC)]
        xg = np.zeros((H, EPC * C), dtype=np.float32)
        widx = np.full((EPC * C, 1), ZERO_ROW_FLAT, dtype=np.int32)
        for j, e in enumerate(es):
            tk = tok_lists[e]
            xg[:, j * C:j * C + len(tk)] = xT[:, tk]
            widx[j * C:j * C + len(tk), 0] = (tk * E + e).astype(np.int32)
        im = {
            "xt": xT, "gwt": gwt, "mask": mask, "xg": xg, "widx": widx,
            "swg": np.ascontiguousarray(sw_gate[:, c * ISS:(c + 1) * ISS]),
            "swu": np.ascontiguousarray(sw_up[:, c * ISS:(c + 1) * ISS]),
            "swd": np.ascontiguousarray(sw_down[c * ISS:(c + 1) * ISS, :]),
        }
        for j, e in enumerate(es):
            im[f"wg{j}"] = w_gate[e]
            im[f"wu{j}"] = w_up[e]
            im[f"wd{j}"] = w_down[e]
        in_maps.append(im)

    trace = bool(int(os.environ.get("BASSMOE_TRACE", "0")))
    kwargs = {}
    if trace:
        kwargs = dict(trace=True, tmpdir=os.environ.get("BASSMOE_TRACE_DIR") or None)
    res = run_bass_kernel_spmd(nc, in_maps, core_ids=list(range(NCORES)), **kwargs)
    LAST_RESULTS = res

    # ---- host: unshard (scatter-add compact expert outputs + sum partials) ----
    y = np.zeros((T, H), dtype=np.float64)
    st_sum = np.zeros((H, T), dtype=np.float64)
    for c in range(NCORES):
        r = res.results[c]
        st_sum += r["st"]
        for j in range(EPC):
            e = EPC * c + j
            tk = tok_lists[e]
            y[tk] += r["zt"][:, j * C:j * C + len(tk)].T
    y += st_sum.T
    return y.astype(np.float32).reshape(orig_shape)
